# revision 25
# baseline (speedup 1.0000x reference)
"""Trainium2 Bass kernel for the CASM MoE-routing wrapper module.

Computation (per batch element b):
    query   = mean_s embeds[b, s, :]                      [H]
    h1      = relu(W1 @ query + b1)                       [RH]
    logits  = W2 @ h1 + b2                                [N]
    top-4 slots + softmax over their logits -> w_full[n] (0 outside top-4)
    contrib = sum_{n,m} w_full[n] * sigmoid(gate[n,m]) * memory[n,m,:]
    out[b]  = hidden[b] + contrib

Sharding: data-parallel over batch. B == n_cores == 8, so core i handles
batch element i end-to-end; the (tiny) router weights and the 16MB slot
bank are replicated on every core. No collectives needed.

Schedule (DMA-bound kernel; HBM streaming is the roofline):
  phase 1: embeds stream (sync ring) + slot-bank stream (gpsimd ring,
           f32->bf16 cast on load). The batch-INDEPENDENT per-slot sum
           all_contribs[n,:] = sum_m sigmoid(gate[n,m])*memory[n,m,:]
           is contracted on the PE as the bank arrives, using per-block
           diagonal'd sigmoid(gate) stationaries (bf16 -> fast weight load).
  phase 2: tiny router MLP + top-4 + softmax (PE/DVE/ACT, ~20us tail).
  phase 3: contrib = w_full @ all_contribs, pre-replicated across all 128
           partitions by broadcasting w_full columns in the stationary.
  phase 4: hidden stream in (sync ring), DVE broadcast-add, store out
           (scalar ring, so stores never head-of-line block loads).

Host-side we only re-layout the replicated weights (transposes/reshapes)
so the device kernel gets natural partition-major DMA patterns:
  w1t    = W1.T                               [H, RH]
  b1t    = b1.reshape(2, 128).T               [128, 2]
  w2tb   = W2.T blocked                       [128, 2*64]
  gate_r = gate_logits.T flattened/blocked    [128, 8]
  mem2   = memory m-major                     [N*MEM, H] row r = m*64 + n
  maskd  = tiled identity                     [128, 64]  (slot diagonal)
"""

import sys

sys.path.insert(0, "/opt/trn_rl_repo")

import numpy as np

import concourse.bass as bass  # noqa: F401  (engine types referenced via nc)
import concourse.mybir as mybir
import concourse.tile as tile
from concourse import bacc
from concourse.bass_utils import run_bass_kernel_spmd

B, S, H = 8, 2048, 4096
N_SLOTS, MEM, RH = 64, 16, 256
TOP_K = 4
P = 128
ST = S // P            # 16 sequence tiles per core
HB = H // P            # 32 H blocks of 128
NCH = H // 512         # 8 matmul free-dim chunks
KB = (N_SLOTS * MEM) // P  # 8 contraction blocks over (slot, mem-row)
RB = RH // P           # 2 router-hidden blocks
DT = mybir.dt.float32
BF = mybir.dt.bfloat16


def build_kernel(tc: tile.TileContext, embeds, hidden, w1t, b1t, w2tb, b2,
                 gate_r, mem2, maskd, out):
    nc = tc.nc
    AF = mybir.ActivationFunctionType  # noqa: N806
    HHALF = H // 2

    with tc.tile_pool(name="cpool", bufs=1) as cpool:
        # --- constants / small replicated weights -------------------------
        ones = cpool.tile([P, 1], DT)
        nc.vector.memset(ones, 1.0 / S)  # folds the mean's 1/S into the colsum
        one11 = cpool.tile([1, 1], DT)
        nc.vector.memset(one11, 1.0)
        w1t_sb = cpool.tile([P, HB * RH], BF)  # bf16 -> PE fast weight load
        nc.scalar.dma_start(
            out=w1t_sb.rearrange("p (hb r) -> p hb r", hb=HB),
            in_=w1t.rearrange("(hb p) r -> p hb r", p=P),
        )
        b1t_sb = cpool.tile([P, RB], DT)
        nc.scalar.dma_start(out=b1t_sb, in_=b1t)
        w2tb_sb = cpool.tile([P, RB * N_SLOTS], DT)
        nc.scalar.dma_start(out=w2tb_sb, in_=w2tb)
        b2_sb = cpool.tile([1, N_SLOTS], DT)
        nc.scalar.dma_start(out=b2_sb, in_=b2)
        gate_sb = cpool.tile([P, KB], DT)
        nc.scalar.dma_start(out=gate_sb, in_=gate_r)
        maskd_sb = cpool.tile([P, N_SLOTS], DT)
        nc.scalar.dma_start(out=maskd_sb, in_=maskd)

        # per-block diagonal'd sigmoid(gate): sgd[kb][p, n] = sig(gate_r[p,kb])
        # iff n == p % 64 else 0  (stationaries for the slot-bank contraction)
        sig_sb = cpool.tile([P, KB], DT)
        nc.scalar.activation(out=sig_sb, in_=gate_sb, func=AF.Sigmoid)
        sgd = cpool.tile([P, KB * N_SLOTS], BF)
        for kb in range(KB):
            nc.vector.tensor_scalar(
                out=sgd[:, kb * N_SLOTS:(kb + 1) * N_SLOTS], in0=maskd_sb,
                scalar1=sig_sb[:, kb:kb + 1], scalar2=None,
                op0=mybir.AluOpType.mult,
            )

        # --- phase 1a: stream the slot bank (f32), contract per-slot sums
        # all_contribs[n, h] = sum_m sigmoid(gate[n, m]) * memory[n, m, h]
        ac_bf = cpool.tile([N_SLOTS, H], BF)
        acc = cpool.tile([P, H], DT)
        last_stream_dma = None
        with tc.tile_pool(name="acp", bufs=1, space="PSUM") as acp:
            psum_ac = acp.tile([N_SLOTS, H], DT)
            with tc.tile_pool(name="mpool", bufs=4) as mpool:
                for kb in range(KB):
                    mt = mpool.tile([P, H], BF)
                    nc.scalar.dma_start(out=mt,
                                        in_=mem2[kb * P:(kb + 1) * P, :])
                    for nch in range(NCH):
                        nc.tensor.matmul(
                            out=psum_ac[:, nch * 512:(nch + 1) * 512],
                            lhsT=sgd[:, kb * N_SLOTS:(kb + 1) * N_SLOTS],
                            rhs=mt[:, nch * 512:(nch + 1) * 512],
                            start=(kb == 0), stop=(kb == KB - 1),
                        )

                # --- phase 1b: stream embeds, accumulate sequence tiles ---
                with tc.tile_pool(name="epool", bufs=3) as epool:
                    for i in range(ST):
                        et = epool.tile([P, H], DT)
                        last_stream_dma = nc.sync.dma_start(
                            out=et, in_=embeds[i * P:(i + 1) * P, :])
                        if i == 0:
                            nc.vector.tensor_copy(out=acc, in_=et)
                        else:
                            nc.vector.tensor_add(out=acc, in0=acc, in1=et)

            nc.vector.tensor_copy(out=ac_bf, in_=psum_ac)

        # --- phase 2: router ---------------------------------------------
        with tc.tile_pool(name="rp", bufs=1, space="PSUM") as rp:
                # qT[p, hb] = query[hb*128 + p] (scaled by 1/S via `ones`)
                psum_qt = rp.tile([P, HB], DT)
                for hb in range(HB):
                    nc.tensor.matmul(
                        out=psum_qt[:, hb:hb + 1],
                        lhsT=acc[:, hb * P:(hb + 1) * P],
                        rhs=ones,
                        start=True, stop=True,
                    )
                qt_bf = cpool.tile([P, HB], BF)
                nc.vector.tensor_copy(out=qt_bf, in_=psum_qt)

                # layer 1: h1T[p, rb] = sum_h W1[rb*128+p, h] * q[h]
                psum_h1t = rp.tile([P, RB], DT)
                for rb in range(RB):
                    for hb in range(HB):
                        nc.tensor.matmul(
                            out=psum_h1t[:, rb:rb + 1],
                            lhsT=w1t_sb[:, (hb * RB + rb) * P:(hb * RB + rb + 1) * P],
                            rhs=qt_bf[:, hb:hb + 1],
                            start=(hb == 0), stop=(hb == HB - 1),
                        )
                h1t_sb = cpool.tile([P, RB], DT)
                nc.vector.tensor_add(out=h1t_sb, in0=psum_h1t, in1=b1t_sb)
                nc.vector.tensor_scalar_max(h1t_sb, h1t_sb, 0.0)

                # layer 2: logits[1, n] = sum_r h1[r] * W2[n, r] (TEMP == 1)
                psum_log = rp.tile([1, N_SLOTS], DT)
                for rb in range(RB):
                    nc.tensor.matmul(
                        out=psum_log,
                        lhsT=h1t_sb[:, rb:rb + 1],
                        rhs=w2tb_sb[:, rb * N_SLOTS:(rb + 1) * N_SLOTS],
                        start=(rb == 0), stop=(rb == RB - 1),
                    )
                logits = cpool.tile([1, N_SLOTS], DT)
                nc.vector.tensor_add(out=logits, in0=psum_log, in1=b2_sb)

                # top-4 + softmax over selected logits, scattered to 64 slots
                max8 = cpool.tile([1, 8], DT)
                nc.vector.max(out=max8, in_=logits)
                negm = cpool.tile([1, 1], DT)
                nc.vector.tensor_scalar_mul(negm, max8[:, 0:1], -1.0)
                e4 = cpool.tile([1, TOP_K], DT)
                zsum = cpool.tile([1, 1], DT)
                nc.scalar.activation(out=e4, in_=max8[:, 0:TOP_K], func=AF.Exp,
                                     bias=negm, accum_out=zsum)
                rz = cpool.tile([1, 1], DT)
                nc.vector.reciprocal(rz, zsum)
                ind = cpool.tile([1, N_SLOTS], DT)
                nc.vector.tensor_scalar(
                    out=ind, in0=logits, scalar1=max8[:, TOP_K - 1:TOP_K],
                    scalar2=None, op0=mybir.AluOpType.is_ge,
                )
                ew = cpool.tile([1, N_SLOTS], DT)
                nc.scalar.activation(out=ew, in_=logits, func=AF.Exp, bias=negm)
                wf = cpool.tile([1, N_SLOTS], DT)
                nc.vector.tensor_tensor(out=wf, in0=ew, in1=ind,
                                        op=mybir.AluOpType.mult)
                nc.vector.tensor_scalar(out=wf, in0=wf, scalar1=rz,
                                        scalar2=None, op0=mybir.AluOpType.mult)

                # w_full onto partitions, then broadcast to a [64, 128]
                # stationary so the combine lands replicated on all partitions
                psum_w = rp.tile([N_SLOTS, 1], DT)
                nc.tensor.matmul(out=psum_w, lhsT=wf, rhs=one11,
                                 start=True, stop=True)
                wf64 = cpool.tile([N_SLOTS, 1], DT)
                nc.vector.tensor_copy(out=wf64, in_=psum_w)
                wf_rep = cpool.tile([N_SLOTS, P], BF)
                nc.vector.tensor_copy(out=wf_rep,
                                      in_=wf64.to_broadcast([N_SLOTS, P]))

        # --- phase 3: contrib = w_full @ all_contribs (replicated) --------
        contrib = cpool.tile([P, H], DT)
        with tc.tile_pool(name="cp", bufs=1, space="PSUM") as cp:
            psum_c = cp.tile([P, H], DT)
            for nch in range(NCH):
                nc.tensor.matmul(
                    out=psum_c[:, nch * 512:(nch + 1) * 512],
                    lhsT=wf_rep,
                    rhs=ac_bf[:, nch * 512:(nch + 1) * 512],
                    start=True, stop=True,
                )
            nc.vector.tensor_copy(out=contrib, in_=psum_c)

        # --- phase 4: stream hidden, broadcast-add contrib, store ---------
        from concourse.tile import add_dep_helper
        with tc.tile_pool(name="spool", bufs=4) as spool:
            for i in range(ST):
                ht = spool.tile([P, H], DT)
                hdma = nc.sync.dma_start(out=ht,
                                         in_=hidden[i * P:(i + 1) * P, :])
                if last_stream_dma is not None:
                    # keep the phase-1 streams ahead of hidden prefetch on
                    # the sync ring (avoids head-of-line blocking embeds)
                    add_dep_helper(hdma.ins, last_stream_dma.ins,
                                   sync=False, reason="phase order")
                nc.vector.tensor_add(out=ht, in0=ht, in1=contrib)
                nc.scalar.dma_start(out=out[i * P:(i + 1) * P, :], in_=ht)


def build_nc(debug: bool = False):
    nc = bacc.Bacc("TRN2", target_bir_lowering=False, debug=debug,
                   num_devices=B)
    embeds = nc.dram_tensor("embeds", [S, H], DT, kind="ExternalInput").ap()
    hidden = nc.dram_tensor("hidden", [S, H], DT, kind="ExternalInput").ap()
    w1t = nc.dram_tensor("w1t", [H, RH], BF, kind="ExternalInput").ap()
    b1t = nc.dram_tensor("b1t", [P, RB], DT, kind="ExternalInput").ap()
    w2tb = nc.dram_tensor("w2tb", [P, RB * N_SLOTS], DT,
                          kind="ExternalInput").ap()
    b2 = nc.dram_tensor("b2", [1, N_SLOTS], DT, kind="ExternalInput").ap()
    gate_r = nc.dram_tensor("gate_r", [P, KB], DT, kind="ExternalInput").ap()
    mem2 = nc.dram_tensor("mem2", [N_SLOTS * MEM, H], BF,
                          kind="ExternalInput").ap()
    maskd = nc.dram_tensor("maskd", [P, N_SLOTS], DT,
                           kind="ExternalInput").ap()
    out = nc.dram_tensor("out", [S, H], DT, kind="ExternalOutput").ap()

    with tile.TileContext(nc) as tc:
        build_kernel(tc, embeds, hidden, w1t, b1t, w2tb, b2, gate_r, mem2,
                     maskd, out)
    nc.compile()
    return nc


_NC_CACHE = None


def _get_nc():
    global _NC_CACHE
    if _NC_CACHE is None:
        _NC_CACHE = build_nc(debug=False)
    return _NC_CACHE


def make_in_maps(inputs: dict) -> list[dict]:
    embeds = np.asarray(inputs["embeds"], dtype=np.float32)
    hidden = np.asarray(inputs["hidden"], dtype=np.float32)
    W1 = np.asarray(inputs["W1"], dtype=np.float32)
    b1 = np.asarray(inputs["b1"], dtype=np.float32)
    W2 = np.asarray(inputs["W2"], dtype=np.float32)
    b2 = np.asarray(inputs["b2"], dtype=np.float32)
    gate_logits = np.asarray(inputs["gate_logits"], dtype=np.float32)
    memory = np.asarray(inputs["memory"], dtype=np.float32)
    import ml_dtypes

    shared = {
        "w1t": np.ascontiguousarray(W1.T).astype(ml_dtypes.bfloat16),
        "b1t": np.ascontiguousarray(b1.reshape(RB, P).T),
        "w2tb": np.ascontiguousarray(
            W2.T.reshape(RB, P, N_SLOTS).transpose(1, 0, 2).reshape(P, RB * N_SLOTS)
        ),
        "b2": np.ascontiguousarray(b2.reshape(1, N_SLOTS)),
        "gate_r": np.ascontiguousarray(gate_logits.T.reshape(KB, P).T),
        "mem2": np.ascontiguousarray(
            memory.transpose(1, 0, 2).reshape(N_SLOTS * MEM, H)
        ).astype(ml_dtypes.bfloat16),
        "maskd": np.ascontiguousarray(
            np.tile(np.eye(N_SLOTS, dtype=np.float32), (P // N_SLOTS, 1))
        ),
    }
    maps = []
    for c in range(B):
        m = dict(shared)
        m["embeds"] = np.ascontiguousarray(embeds[c])
        m["hidden"] = np.ascontiguousarray(hidden[c])
        maps.append(m)
    return maps


def run_spmd(inputs: dict, trace: bool = False):
    """Returns (output [B, S, H] float32, BassKernelResults)."""
    assert int(inputs.get("top_k", TOP_K)) == TOP_K
    nc = _get_nc()
    in_maps = make_in_maps(inputs)
    res = run_bass_kernel_spmd(nc, in_maps, core_ids=list(range(B)),
                               trace=trace)
    outs = np.stack([np.asarray(r["out"], dtype=np.float32)
                     for r in res.results])
    return outs, res


def kernel(**inputs) -> np.ndarray:
    outs, _ = run_spmd(inputs, trace=False)
    return outs


# revision 26
# speedup vs baseline: 1.0010x; 1.0010x over previous
"""Trainium2 Bass kernel for the CASM MoE-routing wrapper module.

Computation (per batch element b):
    query   = mean_s embeds[b, s, :]                      [H]
    h1      = relu(W1 @ query + b1)                       [RH]
    logits  = W2 @ h1 + b2                                [N]
    top-4 slots + softmax over their logits -> w_full[n] (0 outside top-4)
    contrib = sum_{n,m} w_full[n] * sigmoid(gate[n,m]) * memory[n,m,:]
    out[b]  = hidden[b] + contrib

Sharding: data-parallel over batch. B == n_cores == 8, so core i handles
batch element i end-to-end; the (tiny) router weights and the 16MB slot
bank are replicated on every core. No collectives needed.

Schedule (DMA-bound kernel; HBM streaming is the roofline):
  phase 1: embeds stream (sync HWDGE ring) + slot-bank stream (scalar
           HWDGE ring; SWDGE degrades the sync ring, so it is unused).
           The batch-INDEPENDENT per-slot sum
           all_contribs[n,:] = sum_m sigmoid(gate[n,m])*memory[n,m,:]
           is contracted on the PE as the bank arrives, using per-block
           diagonal'd sigmoid(gate) stationaries.
  phase 2: tiny router MLP + top-4 + softmax (PE/DVE/ACT, ~25us tail).
  phase 3: contrib = w_full @ all_contribs, pre-replicated across all 128
           partitions by broadcasting w_full columns in the stationary.
  phase 4: hidden stream in (sync ring), DVE broadcast-add, store out
           (scalar ring, so stores never head-of-line block loads).

Host-side we only re-layout the replicated weights (transposes/reshapes/
bf16 snap for the tensors consumed in bf16 anyway) so the device kernel
gets natural partition-major DMA patterns:
  w1t    = W1.T (bf16)                        [H, RH]
  b1t    = b1.reshape(2, 128).T               [128, 2]
  w2tb   = W2.T blocked                       [128, 2*64]
  gate_r = gate_logits.T flattened/blocked    [128, 8]
  mem2   = memory m-major (bf16)              [N*MEM, H] row r = m*64 + n
  maskd  = tiled identity                     [128, 64]  (slot diagonal)
"""

import sys

sys.path.insert(0, "/opt/trn_rl_repo")

import numpy as np

import concourse.bass as bass  # noqa: F401  (engine types referenced via nc)
import concourse.mybir as mybir
import concourse.tile as tile
from concourse import bacc
from concourse.bass_utils import run_bass_kernel_spmd

B, S, H = 8, 2048, 4096
N_SLOTS, MEM, RH = 64, 16, 256
TOP_K = 4
P = 128
ST = S // P            # 16 sequence tiles per core
HB = H // P            # 32 H blocks of 128
NCH = H // 512         # 8 matmul free-dim chunks
KB = (N_SLOTS * MEM) // P  # 8 contraction blocks over (slot, mem-row)
RB = RH // P           # 2 router-hidden blocks
DT = mybir.dt.float32
BF = mybir.dt.bfloat16


def build_kernel(tc: tile.TileContext, embeds, hidden, w1t, b1t, w2tb, b2,
                 gate_r, mem2, maskd, out):
    nc = tc.nc
    AF = mybir.ActivationFunctionType  # noqa: N806
    HHALF = H // 2

    with tc.tile_pool(name="cpool", bufs=1) as cpool:
        # --- constants / small replicated weights -------------------------
        ones = cpool.tile([P, 1], DT)
        nc.vector.memset(ones, 1.0 / S)  # folds the mean's 1/S into the colsum
        one11 = cpool.tile([1, 1], DT)
        nc.vector.memset(one11, 1.0)
        w1t_sb = cpool.tile([P, HB * RH], BF)  # bf16 -> PE fast weight load
        nc.scalar.dma_start(
            out=w1t_sb.rearrange("p (hb r) -> p hb r", hb=HB),
            in_=w1t.rearrange("(hb p) r -> p hb r", p=P),
        )
        b1t_sb = cpool.tile([P, RB], DT)
        nc.scalar.dma_start(out=b1t_sb, in_=b1t)
        w2tb_sb = cpool.tile([P, RB * N_SLOTS], DT)
        nc.scalar.dma_start(out=w2tb_sb, in_=w2tb)
        b2_sb = cpool.tile([1, N_SLOTS], DT)
        nc.scalar.dma_start(out=b2_sb, in_=b2)
        gate_sb = cpool.tile([P, KB], DT)
        nc.scalar.dma_start(out=gate_sb, in_=gate_r)
        maskd_sb = cpool.tile([P, N_SLOTS], DT)
        nc.scalar.dma_start(out=maskd_sb, in_=maskd)

        # per-block diagonal'd sigmoid(gate): sgd[kb][p, n] = sig(gate_r[p,kb])
        # iff n == p % 64 else 0  (stationaries for the slot-bank contraction)
        sig_sb = cpool.tile([P, KB], DT)
        nc.scalar.activation(out=sig_sb, in_=gate_sb, func=AF.Sigmoid)
        sgd = cpool.tile([P, KB * N_SLOTS], BF)
        for kb in range(KB):
            nc.vector.tensor_scalar(
                out=sgd[:, kb * N_SLOTS:(kb + 1) * N_SLOTS], in0=maskd_sb,
                scalar1=sig_sb[:, kb:kb + 1], scalar2=None,
                op0=mybir.AluOpType.mult,
            )

        # --- phase 1a: stream the slot bank (f32), contract per-slot sums
        # all_contribs[n, h] = sum_m sigmoid(gate[n, m]) * memory[n, m, h]
        ac_bf = cpool.tile([N_SLOTS, H], BF)
        acc = cpool.tile([P, H], DT)
        last_stream_dma = None
        with tc.tile_pool(name="acp", bufs=1, space="PSUM") as acp:
            psum_ac = acp.tile([N_SLOTS, H], DT)
            with tc.tile_pool(name="mpool", bufs=4) as mpool:
                for kb in range(KB):
                    mt = mpool.tile([P, H], BF)
                    nc.scalar.dma_start(out=mt,
                                        in_=mem2[kb * P:(kb + 1) * P, :])
                    for nch in range(NCH):
                        nc.tensor.matmul(
                            out=psum_ac[:, nch * 512:(nch + 1) * 512],
                            lhsT=sgd[:, kb * N_SLOTS:(kb + 1) * N_SLOTS],
                            rhs=mt[:, nch * 512:(nch + 1) * 512],
                            start=(kb == 0), stop=(kb == KB - 1),
                        )

                # --- phase 1b: stream embeds, accumulate sequence tiles ---
                with tc.tile_pool(name="epool", bufs=3) as epool:
                    for i in range(ST):
                        et = epool.tile([P, H], DT)
                        last_stream_dma = nc.sync.dma_start(
                            out=et, in_=embeds[i * P:(i + 1) * P, :])
                        if i == 0:
                            nc.vector.tensor_copy(out=acc, in_=et)
                        else:
                            nc.vector.tensor_add(out=acc, in0=acc, in1=et)

            nc.vector.tensor_copy(out=ac_bf, in_=psum_ac)

        # --- phase 2: router ---------------------------------------------
        with tc.tile_pool(name="rp", bufs=1, space="PSUM") as rp:
                # qT[p, hb] = query[hb*128 + p] (scaled by 1/S via `ones`)
                psum_qt = rp.tile([P, HB], DT)
                for hb in range(HB):
                    nc.tensor.matmul(
                        out=psum_qt[:, hb:hb + 1],
                        lhsT=acc[:, hb * P:(hb + 1) * P],
                        rhs=ones,
                        start=True, stop=True,
                    )
                qt_bf = cpool.tile([P, HB], BF)
                nc.vector.tensor_copy(out=qt_bf, in_=psum_qt)

                # layer 1: h1T[p, rb] = sum_h W1[rb*128+p, h] * q[h]
                psum_h1t = rp.tile([P, RB], DT)
                for rb in range(RB):
                    for hb in range(HB):
                        nc.tensor.matmul(
                            out=psum_h1t[:, rb:rb + 1],
                            lhsT=w1t_sb[:, (hb * RB + rb) * P:(hb * RB + rb + 1) * P],
                            rhs=qt_bf[:, hb:hb + 1],
                            start=(hb == 0), stop=(hb == HB - 1),
                        )
                h1t_sb = cpool.tile([P, RB], DT)
                nc.vector.tensor_add(out=h1t_sb, in0=psum_h1t, in1=b1t_sb)
                nc.vector.tensor_scalar_max(h1t_sb, h1t_sb, 0.0)

                # layer 2: logits[1, n] = sum_r h1[r] * W2[n, r] (TEMP == 1)
                psum_log = rp.tile([1, N_SLOTS], DT)
                for rb in range(RB):
                    nc.tensor.matmul(
                        out=psum_log,
                        lhsT=h1t_sb[:, rb:rb + 1],
                        rhs=w2tb_sb[:, rb * N_SLOTS:(rb + 1) * N_SLOTS],
                        start=(rb == 0), stop=(rb == RB - 1),
                    )
                logits = cpool.tile([1, N_SLOTS], DT)
                nc.vector.tensor_add(out=logits, in0=psum_log, in1=b2_sb)

                # top-4 + softmax over selected logits, scattered to 64 slots
                max8 = cpool.tile([1, 8], DT)
                nc.vector.max(out=max8, in_=logits)
                negm = cpool.tile([1, 1], DT)
                nc.vector.tensor_scalar_mul(negm, max8[:, 0:1], -1.0)
                e4 = cpool.tile([1, TOP_K], DT)
                zsum = cpool.tile([1, 1], DT)
                nc.scalar.activation(out=e4, in_=max8[:, 0:TOP_K], func=AF.Exp,
                                     bias=negm, accum_out=zsum)
                rz = cpool.tile([1, 1], DT)
                nc.vector.reciprocal(rz, zsum)
                ind = cpool.tile([1, N_SLOTS], DT)
                nc.vector.tensor_scalar(
                    out=ind, in0=logits, scalar1=max8[:, TOP_K - 1:TOP_K],
                    scalar2=None, op0=mybir.AluOpType.is_ge,
                )
                ew = cpool.tile([1, N_SLOTS], DT)
                nc.scalar.activation(out=ew, in_=logits, func=AF.Exp, bias=negm)
                wf = cpool.tile([1, N_SLOTS], DT)
                nc.vector.tensor_tensor(out=wf, in0=ew, in1=ind,
                                        op=mybir.AluOpType.mult)
                nc.vector.tensor_scalar(out=wf, in0=wf, scalar1=rz,
                                        scalar2=None, op0=mybir.AluOpType.mult)

                # w_full onto partitions, then broadcast to a [64, 128]
                # stationary so the combine lands replicated on all partitions
                psum_w = rp.tile([N_SLOTS, 1], DT)
                nc.tensor.matmul(out=psum_w, lhsT=wf, rhs=one11,
                                 start=True, stop=True)
                wf64 = cpool.tile([N_SLOTS, 1], DT)
                nc.vector.tensor_copy(out=wf64, in_=psum_w)
                wf_rep = cpool.tile([N_SLOTS, P], BF)
                nc.vector.tensor_copy(out=wf_rep,
                                      in_=wf64.to_broadcast([N_SLOTS, P]))

        # --- phase 3: contrib = w_full @ all_contribs (replicated) --------
        contrib = cpool.tile([P, H], DT)
        with tc.tile_pool(name="cp", bufs=1, space="PSUM") as cp:
            psum_c = cp.tile([P, H], DT)
            for nch in range(NCH):
                nc.tensor.matmul(
                    out=psum_c[:, nch * 512:(nch + 1) * 512],
                    lhsT=wf_rep,
                    rhs=ac_bf[:, nch * 512:(nch + 1) * 512],
                    start=True, stop=True,
                )
            nc.vector.tensor_copy(out=contrib, in_=psum_c)

        # --- phase 4: stream hidden, broadcast-add contrib, store ---------
        from concourse.tile import add_dep_helper
        with tc.tile_pool(name="spool", bufs=4) as spool:
            for i in range(ST):
                ht = spool.tile([P, H], DT)
                hdma = nc.sync.dma_start(out=ht,
                                         in_=hidden[i * P:(i + 1) * P, :])
                if last_stream_dma is not None:
                    # keep the phase-1 streams ahead of hidden prefetch on
                    # the sync ring (avoids head-of-line blocking embeds)
                    add_dep_helper(hdma.ins, last_stream_dma.ins,
                                   sync=False, reason="phase order")
                nc.vector.tensor_add(out=ht, in0=ht, in1=contrib)
                nc.scalar.dma_start(out=out[i * P:(i + 1) * P, :], in_=ht)


def build_nc(debug: bool = False):
    nc = bacc.Bacc("TRN2", target_bir_lowering=False, debug=debug,
                   num_devices=B)
    embeds = nc.dram_tensor("embeds", [S, H], DT, kind="ExternalInput").ap()
    hidden = nc.dram_tensor("hidden", [S, H], DT, kind="ExternalInput").ap()
    w1t = nc.dram_tensor("w1t", [H, RH], BF, kind="ExternalInput").ap()
    b1t = nc.dram_tensor("b1t", [P, RB], DT, kind="ExternalInput").ap()
    w2tb = nc.dram_tensor("w2tb", [P, RB * N_SLOTS], DT,
                          kind="ExternalInput").ap()
    b2 = nc.dram_tensor("b2", [1, N_SLOTS], DT, kind="ExternalInput").ap()
    gate_r = nc.dram_tensor("gate_r", [P, KB], DT, kind="ExternalInput").ap()
    mem2 = nc.dram_tensor("mem2", [N_SLOTS * MEM, H], BF,
                          kind="ExternalInput").ap()
    maskd = nc.dram_tensor("maskd", [P, N_SLOTS], DT,
                           kind="ExternalInput").ap()
    out = nc.dram_tensor("out", [S, H], DT, kind="ExternalOutput").ap()

    with tile.TileContext(nc) as tc:
        build_kernel(tc, embeds, hidden, w1t, b1t, w2tb, b2, gate_r, mem2,
                     maskd, out)
    nc.compile()
    return nc


_NC_CACHE = None


def _get_nc():
    global _NC_CACHE
    if _NC_CACHE is None:
        _NC_CACHE = build_nc(debug=False)
    return _NC_CACHE


def make_in_maps(inputs: dict) -> list[dict]:
    embeds = np.asarray(inputs["embeds"], dtype=np.float32)
    hidden = np.asarray(inputs["hidden"], dtype=np.float32)
    W1 = np.asarray(inputs["W1"], dtype=np.float32)
    b1 = np.asarray(inputs["b1"], dtype=np.float32)
    W2 = np.asarray(inputs["W2"], dtype=np.float32)
    b2 = np.asarray(inputs["b2"], dtype=np.float32)
    gate_logits = np.asarray(inputs["gate_logits"], dtype=np.float32)
    memory = np.asarray(inputs["memory"], dtype=np.float32)
    import ml_dtypes

    shared = {
        "w1t": np.ascontiguousarray(W1.T).astype(ml_dtypes.bfloat16),
        "b1t": np.ascontiguousarray(b1.reshape(RB, P).T),
        "w2tb": np.ascontiguousarray(
            W2.T.reshape(RB, P, N_SLOTS).transpose(1, 0, 2).reshape(P, RB * N_SLOTS)
        ),
        "b2": np.ascontiguousarray(b2.reshape(1, N_SLOTS)),
        "gate_r": np.ascontiguousarray(gate_logits.T.reshape(KB, P).T),
        "mem2": np.ascontiguousarray(
            memory.transpose(1, 0, 2).reshape(N_SLOTS * MEM, H)
        ).astype(ml_dtypes.bfloat16),
        "maskd": np.ascontiguousarray(
            np.tile(np.eye(N_SLOTS, dtype=np.float32), (P // N_SLOTS, 1))
        ),
    }
    maps = []
    for c in range(B):
        m = dict(shared)
        m["embeds"] = np.ascontiguousarray(embeds[c])
        m["hidden"] = np.ascontiguousarray(hidden[c])
        maps.append(m)
    return maps


def run_spmd(inputs: dict, trace: bool = False):
    """Returns (output [B, S, H] float32, BassKernelResults)."""
    assert int(inputs.get("top_k", TOP_K)) == TOP_K
    nc = _get_nc()
    in_maps = make_in_maps(inputs)
    res = run_bass_kernel_spmd(nc, in_maps, core_ids=list(range(B)),
                               trace=trace)
    outs = np.stack([np.asarray(r["out"], dtype=np.float32)
                     for r in res.results])
    return outs, res


def kernel(**inputs) -> np.ndarray:
    outs, _ = run_spmd(inputs, trace=False)
    return outs


# revision 27
# speedup vs baseline: 1.0142x; 1.0132x over previous
"""Trainium2 Bass kernel for the CASM MoE-routing wrapper module.

Computation (per batch element b):
    query   = mean_s embeds[b, s, :]                      [H]
    h1      = relu(W1 @ query + b1)                       [RH]
    logits  = W2 @ h1 + b2                                [N]
    top-4 slots + softmax over their logits -> w_full[n] (0 outside top-4)
    contrib = sum_{n,m} w_full[n] * sigmoid(gate[n,m]) * memory[n,m,:]
    out[b]  = hidden[b] + contrib

Sharding: data-parallel over batch. B == n_cores == 8, so core i handles
batch element i end-to-end; the (tiny) router weights and the 16MB slot
bank are replicated on every core. No collectives needed.

Schedule (DMA-bound kernel; HBM streaming is the roofline):
  phase 1: embeds stream (sync HWDGE ring) + slot-bank stream (scalar
           HWDGE ring; SWDGE degrades the sync ring, so it is unused).
           The batch-INDEPENDENT per-slot sum
           all_contribs[n,:] = sum_m sigmoid(gate[n,m])*memory[n,m,:]
           is contracted on the PE as the bank arrives, using per-block
           diagonal'd sigmoid(gate) stationaries.
  phase 2: tiny router MLP + top-4 + softmax (PE/DVE/ACT, ~25us tail).
  phase 3: contrib = w_full @ all_contribs, pre-replicated across all 128
           partitions by broadcasting w_full columns in the stationary.
  phase 4: hidden stream in (sync ring), DVE broadcast-add, store out
           (scalar ring, so stores never head-of-line block loads).

Host-side we only re-layout the replicated weights (transposes/reshapes/
bf16 snap for the tensors consumed in bf16 anyway) so the device kernel
gets natural partition-major DMA patterns:
  w1t    = W1.T (bf16)                        [H, RH]
  b1t    = b1.reshape(2, 128).T               [128, 2]
  w2tb   = W2.T blocked                       [128, 2*64]
  gate_r = gate_logits.T flattened/blocked    [128, 8]
  mem2   = memory m-major (bf16)              [N*MEM, H] row r = m*64 + n
  maskd  = tiled identity                     [128, 64]  (slot diagonal)
"""

import sys

sys.path.insert(0, "/opt/trn_rl_repo")

import numpy as np

import concourse.bass as bass  # noqa: F401  (engine types referenced via nc)
import concourse.mybir as mybir
import concourse.tile as tile
from concourse import bacc
from concourse.bass_utils import run_bass_kernel_spmd

B, S, H = 8, 2048, 4096
N_SLOTS, MEM, RH = 64, 16, 256
TOP_K = 4
P = 128
ST = S // P            # 16 sequence tiles per core
HB = H // P            # 32 H blocks of 128
NCH = H // 512         # 8 matmul free-dim chunks
KB = (N_SLOTS * MEM) // P  # 8 contraction blocks over (slot, mem-row)
RB = RH // P           # 2 router-hidden blocks
DT = mybir.dt.float32
BF = mybir.dt.bfloat16


def build_kernel(tc: tile.TileContext, embeds, hidden, w1t, b1t, w2tb, b2,
                 gate_r, mem2, maskd, out):
    nc = tc.nc
    AF = mybir.ActivationFunctionType  # noqa: N806
    HHALF = H // 2

    with tc.tile_pool(name="cpool", bufs=1) as cpool:
        # --- constants / small replicated weights -------------------------
        ones = cpool.tile([P, 1], DT)
        nc.vector.memset(ones, 1.0 / S)  # folds the mean's 1/S into the colsum
        one11 = cpool.tile([1, 1], DT)
        nc.vector.memset(one11, 1.0)
        w1t_sb = cpool.tile([P, HB * RH], BF)  # bf16 -> PE fast weight load
        nc.scalar.dma_start(
            out=w1t_sb.rearrange("p (hb r) -> p hb r", hb=HB),
            in_=w1t.rearrange("(hb p) r -> p hb r", p=P),
        )
        b1t_sb = cpool.tile([P, RB], DT)
        nc.scalar.dma_start(out=b1t_sb, in_=b1t)
        w2tb_sb = cpool.tile([P, RB * N_SLOTS], DT)
        nc.scalar.dma_start(out=w2tb_sb, in_=w2tb)
        b2_sb = cpool.tile([1, N_SLOTS], DT)
        nc.scalar.dma_start(out=b2_sb, in_=b2)
        gate_sb = cpool.tile([P, KB], DT)
        nc.scalar.dma_start(out=gate_sb, in_=gate_r)
        maskd_sb = cpool.tile([P, N_SLOTS], DT)
        nc.scalar.dma_start(out=maskd_sb, in_=maskd)

        # per-block diagonal'd sigmoid(gate): sgd[kb][p, n] = sig(gate_r[p,kb])
        # iff n == p % 64 else 0  (stationaries for the slot-bank contraction)
        sig_sb = cpool.tile([P, KB], DT)
        nc.scalar.activation(out=sig_sb, in_=gate_sb, func=AF.Sigmoid)
        sgd = cpool.tile([P, KB * N_SLOTS], BF)
        for kb in range(KB):
            nc.vector.tensor_scalar(
                out=sgd[:, kb * N_SLOTS:(kb + 1) * N_SLOTS], in0=maskd_sb,
                scalar1=sig_sb[:, kb:kb + 1], scalar2=None,
                op0=mybir.AluOpType.mult,
            )

        # --- phase 1a: stream the slot bank (f32), contract per-slot sums
        # all_contribs[n, h] = sum_m sigmoid(gate[n, m]) * memory[n, m, h]
        ac_bf = cpool.tile([N_SLOTS, H], BF)
        acc = cpool.tile([P, H], DT)
        last_stream_dma = None
        with tc.tile_pool(name="acp", bufs=1, space="PSUM") as acp:
            psum_ac = acp.tile([N_SLOTS, H], DT)
            with tc.tile_pool(name="mpool", bufs=4) as mpool:
                for kb in range(KB):
                    mt = mpool.tile([P, H], BF)
                    nc.scalar.dma_start(out=mt,
                                        in_=mem2[kb * P:(kb + 1) * P, :])
                    for nch in range(NCH):
                        nc.tensor.matmul(
                            out=psum_ac[:, nch * 512:(nch + 1) * 512],
                            lhsT=sgd[:, kb * N_SLOTS:(kb + 1) * N_SLOTS],
                            rhs=mt[:, nch * 512:(nch + 1) * 512],
                            start=(kb == 0), stop=(kb == KB - 1),
                        )

                # --- phase 1b: stream embeds, accumulate sequence tiles ---
                with tc.tile_pool(name="epool", bufs=3) as epool:
                    for i in range(ST):
                        et = epool.tile([P, H], DT)
                        last_stream_dma = nc.sync.dma_start(
                            out=et, in_=embeds[i * P:(i + 1) * P, :])
                        if i == 0:
                            nc.vector.tensor_copy(out=acc, in_=et)
                        else:
                            nc.vector.tensor_add(out=acc, in0=acc, in1=et)

            # ac_bf = psum_ac, expressed as (acc*0 + psum_ac) so the copy
            # carries a data dependency on the finished embeds accumulation:
            # the Tile scheduler then cannot place it mid-chain on DVE, where
            # its PSUM wait would head-of-line block the embeds adds
            nc.vector.scalar_tensor_tensor(
                out=ac_bf, in0=acc[0:N_SLOTS, :], scalar=0.0, in1=psum_ac,
                op0=mybir.AluOpType.mult, op1=mybir.AluOpType.add)

        # --- phase 2: router ---------------------------------------------
        with tc.tile_pool(name="rp", bufs=1, space="PSUM") as rp:
                # qT[p, hb] = query[hb*128 + p] (scaled by 1/S via `ones`)
                psum_qt = rp.tile([P, HB], DT)
                for hb in range(HB):
                    nc.tensor.matmul(
                        out=psum_qt[:, hb:hb + 1],
                        lhsT=acc[:, hb * P:(hb + 1) * P],
                        rhs=ones,
                        start=True, stop=True,
                    )
                qt_bf = cpool.tile([P, HB], BF)
                nc.vector.tensor_copy(out=qt_bf, in_=psum_qt)

                # layer 1: h1T[p, rb] = sum_h W1[rb*128+p, h] * q[h]
                psum_h1t = rp.tile([P, RB], DT)
                for rb in range(RB):
                    for hb in range(HB):
                        nc.tensor.matmul(
                            out=psum_h1t[:, rb:rb + 1],
                            lhsT=w1t_sb[:, (hb * RB + rb) * P:(hb * RB + rb + 1) * P],
                            rhs=qt_bf[:, hb:hb + 1],
                            start=(hb == 0), stop=(hb == HB - 1),
                        )
                h1t_sb = cpool.tile([P, RB], DT)
                nc.vector.tensor_add(out=h1t_sb, in0=psum_h1t, in1=b1t_sb)
                nc.vector.tensor_scalar_max(h1t_sb, h1t_sb, 0.0)

                # layer 2: logits[1, n] = sum_r h1[r] * W2[n, r] (TEMP == 1)
                psum_log = rp.tile([1, N_SLOTS], DT)
                for rb in range(RB):
                    nc.tensor.matmul(
                        out=psum_log,
                        lhsT=h1t_sb[:, rb:rb + 1],
                        rhs=w2tb_sb[:, rb * N_SLOTS:(rb + 1) * N_SLOTS],
                        start=(rb == 0), stop=(rb == RB - 1),
                    )
                logits = cpool.tile([1, N_SLOTS], DT)
                nc.vector.tensor_add(out=logits, in0=psum_log, in1=b2_sb)

                # top-4 + softmax over selected logits, scattered to 64 slots
                max8 = cpool.tile([1, 8], DT)
                nc.vector.max(out=max8, in_=logits)
                negm = cpool.tile([1, 1], DT)
                nc.vector.tensor_scalar_mul(negm, max8[:, 0:1], -1.0)
                e4 = cpool.tile([1, TOP_K], DT)
                zsum = cpool.tile([1, 1], DT)
                nc.scalar.activation(out=e4, in_=max8[:, 0:TOP_K], func=AF.Exp,
                                     bias=negm, accum_out=zsum)
                rz = cpool.tile([1, 1], DT)
                nc.vector.reciprocal(rz, zsum)
                ind = cpool.tile([1, N_SLOTS], DT)
                nc.vector.tensor_scalar(
                    out=ind, in0=logits, scalar1=max8[:, TOP_K - 1:TOP_K],
                    scalar2=None, op0=mybir.AluOpType.is_ge,
                )
                ew = cpool.tile([1, N_SLOTS], DT)
                nc.scalar.activation(out=ew, in_=logits, func=AF.Exp, bias=negm)
                wf = cpool.tile([1, N_SLOTS], DT)
                nc.vector.tensor_tensor(out=wf, in0=ew, in1=ind,
                                        op=mybir.AluOpType.mult)
                nc.vector.tensor_scalar(out=wf, in0=wf, scalar1=rz,
                                        scalar2=None, op0=mybir.AluOpType.mult)

                # w_full onto partitions, then broadcast to a [64, 128]
                # stationary so the combine lands replicated on all partitions
                psum_w = rp.tile([N_SLOTS, 1], DT)
                nc.tensor.matmul(out=psum_w, lhsT=wf, rhs=one11,
                                 start=True, stop=True)
                wf64 = cpool.tile([N_SLOTS, 1], DT)
                nc.vector.tensor_copy(out=wf64, in_=psum_w)
                wf_rep = cpool.tile([N_SLOTS, P], BF)
                nc.vector.tensor_copy(out=wf_rep,
                                      in_=wf64.to_broadcast([N_SLOTS, P]))

        # --- phase 3: contrib = w_full @ all_contribs (replicated) --------
        contrib = cpool.tile([P, H], DT)
        with tc.tile_pool(name="cp", bufs=1, space="PSUM") as cp:
            psum_c = cp.tile([P, H], DT)
            for nch in range(NCH):
                nc.tensor.matmul(
                    out=psum_c[:, nch * 512:(nch + 1) * 512],
                    lhsT=wf_rep,
                    rhs=ac_bf[:, nch * 512:(nch + 1) * 512],
                    start=True, stop=True,
                )
            nc.vector.tensor_copy(out=contrib, in_=psum_c)

        # --- phase 4: stream hidden, broadcast-add contrib, store ---------
        from concourse.tile import add_dep_helper
        with tc.tile_pool(name="spool", bufs=4) as spool:
            for i in range(ST):
                ht = spool.tile([P, H], DT)
                hdma = nc.sync.dma_start(out=ht,
                                         in_=hidden[i * P:(i + 1) * P, :])
                if last_stream_dma is not None:
                    # keep the phase-1 streams ahead of hidden prefetch on
                    # the sync ring (avoids head-of-line blocking embeds)
                    add_dep_helper(hdma.ins, last_stream_dma.ins,
                                   sync=False, reason="phase order")
                nc.vector.tensor_add(out=ht, in0=ht, in1=contrib)
                nc.scalar.dma_start(out=out[i * P:(i + 1) * P, :], in_=ht)


def build_nc(debug: bool = False):
    nc = bacc.Bacc("TRN2", target_bir_lowering=False, debug=debug,
                   num_devices=B)
    embeds = nc.dram_tensor("embeds", [S, H], DT, kind="ExternalInput").ap()
    hidden = nc.dram_tensor("hidden", [S, H], DT, kind="ExternalInput").ap()
    w1t = nc.dram_tensor("w1t", [H, RH], BF, kind="ExternalInput").ap()
    b1t = nc.dram_tensor("b1t", [P, RB], DT, kind="ExternalInput").ap()
    w2tb = nc.dram_tensor("w2tb", [P, RB * N_SLOTS], DT,
                          kind="ExternalInput").ap()
    b2 = nc.dram_tensor("b2", [1, N_SLOTS], DT, kind="ExternalInput").ap()
    gate_r = nc.dram_tensor("gate_r", [P, KB], DT, kind="ExternalInput").ap()
    mem2 = nc.dram_tensor("mem2", [N_SLOTS * MEM, H], BF,
                          kind="ExternalInput").ap()
    maskd = nc.dram_tensor("maskd", [P, N_SLOTS], DT,
                           kind="ExternalInput").ap()
    out = nc.dram_tensor("out", [S, H], DT, kind="ExternalOutput").ap()

    with tile.TileContext(nc) as tc:
        build_kernel(tc, embeds, hidden, w1t, b1t, w2tb, b2, gate_r, mem2,
                     maskd, out)
    nc.compile()
    return nc


_NC_CACHE = None


def _get_nc():
    global _NC_CACHE
    if _NC_CACHE is None:
        _NC_CACHE = build_nc(debug=False)
    return _NC_CACHE


def make_in_maps(inputs: dict) -> list[dict]:
    embeds = np.asarray(inputs["embeds"], dtype=np.float32)
    hidden = np.asarray(inputs["hidden"], dtype=np.float32)
    W1 = np.asarray(inputs["W1"], dtype=np.float32)
    b1 = np.asarray(inputs["b1"], dtype=np.float32)
    W2 = np.asarray(inputs["W2"], dtype=np.float32)
    b2 = np.asarray(inputs["b2"], dtype=np.float32)
    gate_logits = np.asarray(inputs["gate_logits"], dtype=np.float32)
    memory = np.asarray(inputs["memory"], dtype=np.float32)
    import ml_dtypes

    shared = {
        "w1t": np.ascontiguousarray(W1.T).astype(ml_dtypes.bfloat16),
        "b1t": np.ascontiguousarray(b1.reshape(RB, P).T),
        "w2tb": np.ascontiguousarray(
            W2.T.reshape(RB, P, N_SLOTS).transpose(1, 0, 2).reshape(P, RB * N_SLOTS)
        ),
        "b2": np.ascontiguousarray(b2.reshape(1, N_SLOTS)),
        "gate_r": np.ascontiguousarray(gate_logits.T.reshape(KB, P).T),
        "mem2": np.ascontiguousarray(
            memory.transpose(1, 0, 2).reshape(N_SLOTS * MEM, H)
        ).astype(ml_dtypes.bfloat16),
        "maskd": np.ascontiguousarray(
            np.tile(np.eye(N_SLOTS, dtype=np.float32), (P // N_SLOTS, 1))
        ),
    }
    maps = []
    for c in range(B):
        m = dict(shared)
        m["embeds"] = np.ascontiguousarray(embeds[c])
        m["hidden"] = np.ascontiguousarray(hidden[c])
        maps.append(m)
    return maps


def run_spmd(inputs: dict, trace: bool = False):
    """Returns (output [B, S, H] float32, BassKernelResults)."""
    assert int(inputs.get("top_k", TOP_K)) == TOP_K
    nc = _get_nc()
    in_maps = make_in_maps(inputs)
    res = run_bass_kernel_spmd(nc, in_maps, core_ids=list(range(B)),
                               trace=trace)
    outs = np.stack([np.asarray(r["out"], dtype=np.float32)
                     for r in res.results])
    return outs, res


def kernel(**inputs) -> np.ndarray:
    outs, _ = run_spmd(inputs, trace=False)
    return outs


# revision 28
# speedup vs baseline: 1.2161x; 1.1991x over previous
"""Trainium2 Bass kernel for the CASM MoE-routing wrapper module.

Computation (per batch element b):
    query   = mean_s embeds[b, s, :]                      [H]
    h1      = relu(W1 @ query + b1)                       [RH]
    logits  = W2 @ h1 + b2                                [N]
    top-4 slots + softmax over their logits -> w_full[n] (0 outside top-4)
    contrib = sum_{n,m} w_full[n] * sigmoid(gate[n,m]) * memory[n,m,:]
    out[b]  = hidden[b] + contrib

Sharding: data-parallel over batch. B == n_cores == 8, so core i handles
batch element i end-to-end; the (tiny) router weights and the 16MB slot
bank are replicated on every core. No collectives needed.

Schedule (DMA-bound kernel; HBM streaming is the roofline):
  phase 1: embeds stream (sync HWDGE ring) + slot-bank stream (scalar
           HWDGE ring; SWDGE degrades the sync ring, so it is unused).
           The batch-INDEPENDENT per-slot sum
           all_contribs[n,:] = sum_m sigmoid(gate[n,m])*memory[n,m,:]
           is contracted on the PE as the bank arrives, using per-block
           diagonal'd sigmoid(gate) stationaries.
  phase 2: tiny router MLP + top-4 + softmax (PE/DVE/ACT, ~25us tail).
  phase 3: contrib = w_full @ all_contribs, pre-replicated across all 128
           partitions by broadcasting w_full columns in the stationary.
  phase 4: hidden stream in (sync ring), DVE broadcast-add, store out
           (scalar ring, so stores never head-of-line block loads).

Host-side we only re-layout the replicated weights (transposes/reshapes/
bf16 snap for the tensors consumed in bf16 anyway) so the device kernel
gets natural partition-major DMA patterns:
  w1t    = W1.T (bf16)                        [H, RH]
  b1t    = b1.reshape(2, 128).T               [128, 2]
  w2tb   = W2.T blocked                       [128, 2*64]
  gate_r = gate_logits.T flattened/blocked    [128, 8]
  mem2   = memory m-major (bf16)              [N*MEM, H] row r = m*64 + n
  maskd  = tiled identity                     [128, 64]  (slot diagonal)
"""

import sys

sys.path.insert(0, "/opt/trn_rl_repo")

import numpy as np

import concourse.bass as bass  # noqa: F401  (engine types referenced via nc)
import concourse.mybir as mybir
import concourse.tile as tile
from concourse import bacc
from concourse.bass_utils import run_bass_kernel_spmd

B, S, H = 8, 2048, 4096
N_SLOTS, MEM, RH = 64, 16, 256
TOP_K = 4
P = 128
ST = S // P            # 16 sequence tiles per core
HB = H // P            # 32 H blocks of 128
NCH = H // 512         # 8 matmul free-dim chunks
KB = (N_SLOTS * MEM) // P  # 8 contraction blocks over (slot, mem-row)
RB = RH // P           # 2 router-hidden blocks
DT = mybir.dt.float32
BF = mybir.dt.bfloat16


def build_kernel(tc: tile.TileContext, embeds, hidden, w1t, b1t, w2tb, b2,
                 gate_r, mem2, maskd, out):
    nc = tc.nc
    AF = mybir.ActivationFunctionType  # noqa: N806
    HHALF = H // 2

    with tc.tile_pool(name="cpool", bufs=1) as cpool:
        # --- constants / small replicated weights -------------------------
        ones = cpool.tile([P, 1], DT)
        nc.vector.memset(ones, 1.0 / S)  # folds the mean's 1/S into the colsum
        one11 = cpool.tile([1, 1], DT)
        nc.vector.memset(one11, 1.0)
        w1t_sb = cpool.tile([P, HB * RH], BF)  # bf16 -> PE fast weight load
        nc.scalar.dma_start(
            out=w1t_sb.rearrange("p (hb r) -> p hb r", hb=HB),
            in_=w1t.rearrange("(hb p) r -> p hb r", p=P),
        )
        b1t_sb = cpool.tile([P, RB], DT)
        nc.scalar.dma_start(out=b1t_sb, in_=b1t)
        w2tb_sb = cpool.tile([P, RB * N_SLOTS], DT)
        nc.scalar.dma_start(out=w2tb_sb, in_=w2tb)
        b2_sb = cpool.tile([1, N_SLOTS], DT)
        nc.scalar.dma_start(out=b2_sb, in_=b2)
        gate_sb = cpool.tile([P, KB], DT)
        nc.scalar.dma_start(out=gate_sb, in_=gate_r)
        maskd_sb = cpool.tile([P, N_SLOTS], DT)
        nc.scalar.dma_start(out=maskd_sb, in_=maskd)

        # per-block diagonal'd sigmoid(gate): sgd[kb][p, n] = sig(gate_r[p,kb])
        # iff n == p % 64 else 0  (stationaries for the slot-bank contraction)
        sig_sb = cpool.tile([P, KB], DT)
        nc.scalar.activation(out=sig_sb, in_=gate_sb, func=AF.Sigmoid)
        sgd = cpool.tile([P, KB * N_SLOTS], BF)
        for kb in range(KB):
            nc.vector.tensor_scalar(
                out=sgd[:, kb * N_SLOTS:(kb + 1) * N_SLOTS], in0=maskd_sb,
                scalar1=sig_sb[:, kb:kb + 1], scalar2=None,
                op0=mybir.AluOpType.mult,
            )

        # --- phase 1a: stream the slot bank (f32), contract per-slot sums
        # all_contribs[n, h] = sum_m sigmoid(gate[n, m]) * memory[n, m, h]
        ac_bf = cpool.tile([N_SLOTS, H], BF)
        acc = cpool.tile([P, H], DT)
        last_stream_dma = None
        with tc.tile_pool(name="acp", bufs=1, space="PSUM") as acp:
            psum_ac = acp.tile([N_SLOTS, H], DT)
            with tc.tile_pool(name="mpool", bufs=4) as mpool:
                for kb in range(KB):
                    mt = mpool.tile([P, H], BF)
                    nc.scalar.dma_start(out=mt,
                                        in_=mem2[kb * P:(kb + 1) * P, :])
                    for nch in range(NCH):
                        nc.tensor.matmul(
                            out=psum_ac[:, nch * 512:(nch + 1) * 512],
                            lhsT=sgd[:, kb * N_SLOTS:(kb + 1) * N_SLOTS],
                            rhs=mt[:, nch * 512:(nch + 1) * 512],
                            start=(kb == 0), stop=(kb == KB - 1),
                        )

                # --- phase 1b: stream embeds, accumulate sequence tiles ---
                with tc.tile_pool(name="epool", bufs=3) as epool:
                    for i in range(ST):
                        et = epool.tile([P, H], DT)
                        last_stream_dma = nc.sync.dma_start(
                            out=et, in_=embeds[i * P:(i + 1) * P, :])
                        if i == 0:
                            nc.vector.tensor_copy(out=acc, in_=et)
                        else:
                            nc.vector.tensor_add(out=acc, in0=acc, in1=et)

            # ac_bf = psum_ac, expressed as (acc*0 + psum_ac) so the copy
            # carries a data dependency on the finished embeds accumulation:
            # the Tile scheduler then cannot place it mid-chain on DVE, where
            # its PSUM wait would head-of-line block the embeds adds
            nc.vector.scalar_tensor_tensor(
                out=ac_bf, in0=acc[0:N_SLOTS, :], scalar=0.0, in1=psum_ac,
                op0=mybir.AluOpType.mult, op1=mybir.AluOpType.add)

        # --- phase 2: router ---------------------------------------------
        with tc.tile_pool(name="rp", bufs=1, space="PSUM") as rp:
                # qT[p, hb] = query[hb*128 + p] (scaled by 1/S via `ones`)
                psum_qt = rp.tile([P, HB], DT)
                for hb in range(HB):
                    nc.tensor.matmul(
                        out=psum_qt[:, hb:hb + 1],
                        lhsT=acc[:, hb * P:(hb + 1) * P],
                        rhs=ones,
                        start=True, stop=True,
                    )
                qt_bf = cpool.tile([P, HB], BF)
                nc.vector.tensor_copy(out=qt_bf, in_=psum_qt)

                # layer 1: h1T[p, rb] = sum_h W1[rb*128+p, h] * q[h]
                psum_h1t = rp.tile([P, RB], DT)
                for rb in range(RB):
                    for hb in range(HB):
                        nc.tensor.matmul(
                            out=psum_h1t[:, rb:rb + 1],
                            lhsT=w1t_sb[:, (hb * RB + rb) * P:(hb * RB + rb + 1) * P],
                            rhs=qt_bf[:, hb:hb + 1],
                            start=(hb == 0), stop=(hb == HB - 1),
                        )
                h1t_sb = cpool.tile([P, RB], DT)
                nc.vector.tensor_add(out=h1t_sb, in0=psum_h1t, in1=b1t_sb)
                nc.vector.tensor_scalar_max(h1t_sb, h1t_sb, 0.0)

                # layer 2: logits[1, n] = sum_r h1[r] * W2[n, r] (TEMP == 1)
                psum_log = rp.tile([1, N_SLOTS], DT)
                for rb in range(RB):
                    nc.tensor.matmul(
                        out=psum_log,
                        lhsT=h1t_sb[:, rb:rb + 1],
                        rhs=w2tb_sb[:, rb * N_SLOTS:(rb + 1) * N_SLOTS],
                        start=(rb == 0), stop=(rb == RB - 1),
                    )
                logits = cpool.tile([1, N_SLOTS], DT)
                nc.vector.tensor_add(out=logits, in0=psum_log, in1=b2_sb)

                # top-4 + softmax over selected logits, scattered to 64 slots
                max8 = cpool.tile([1, 8], DT)
                nc.vector.max(out=max8, in_=logits)
                negm = cpool.tile([1, 1], DT)
                nc.vector.tensor_scalar_mul(negm, max8[:, 0:1], -1.0)
                e4 = cpool.tile([1, TOP_K], DT)
                zsum = cpool.tile([1, 1], DT)
                nc.scalar.activation(out=e4, in_=max8[:, 0:TOP_K], func=AF.Exp,
                                     bias=negm, accum_out=zsum)
                rz = cpool.tile([1, 1], DT)
                nc.vector.reciprocal(rz, zsum)
                ind = cpool.tile([1, N_SLOTS], DT)
                nc.vector.tensor_scalar(
                    out=ind, in0=logits, scalar1=max8[:, TOP_K - 1:TOP_K],
                    scalar2=None, op0=mybir.AluOpType.is_ge,
                )
                ew = cpool.tile([1, N_SLOTS], DT)
                nc.scalar.activation(out=ew, in_=logits, func=AF.Exp, bias=negm)
                wf = cpool.tile([1, N_SLOTS], DT)
                nc.vector.tensor_tensor(out=wf, in0=ew, in1=ind,
                                        op=mybir.AluOpType.mult)
                nc.vector.tensor_scalar(out=wf, in0=wf, scalar1=rz,
                                        scalar2=None, op0=mybir.AluOpType.mult)

                # w_full onto partitions, then broadcast to a [64, 128]
                # stationary so the combine lands replicated on all partitions
                psum_w = rp.tile([N_SLOTS, 1], DT)
                nc.tensor.matmul(out=psum_w, lhsT=wf, rhs=one11,
                                 start=True, stop=True)
                wf64 = cpool.tile([N_SLOTS, 1], DT)
                nc.vector.tensor_copy(out=wf64, in_=psum_w)
                wf_rep = cpool.tile([N_SLOTS, P], BF)
                nc.vector.tensor_copy(out=wf_rep,
                                      in_=wf64.to_broadcast([N_SLOTS, P]))

        # --- phase 3: contrib = w_full @ all_contribs (replicated).
        # Stays in PSUM: the phase-4 adds read it directly as in1, so no
        # PSUM->SBUF copy sits on the critical path.
        from concourse.tile import add_dep_helper
        with tc.tile_pool(name="cp", bufs=1, space="PSUM") as cp, \
                tc.tile_pool(name="spool", bufs=4) as spool:
            psum_c = cp.tile([P, H], DT)
            for nch in range(NCH):
                nc.tensor.matmul(
                    out=psum_c[:, nch * 512:(nch + 1) * 512],
                    lhsT=wf_rep,
                    rhs=ac_bf[:, nch * 512:(nch + 1) * 512],
                    start=True, stop=True,
                )

            # --- phase 4: stream hidden, broadcast-add contrib, store -----
            for i in range(ST):
                ht = spool.tile([P, H], DT)
                hdma = nc.sync.dma_start(out=ht,
                                         in_=hidden[i * P:(i + 1) * P, :])
                if last_stream_dma is not None:
                    # keep the phase-1 streams ahead of hidden prefetch on
                    # the sync ring (avoids head-of-line blocking embeds)
                    add_dep_helper(hdma.ins, last_stream_dma.ins,
                                   sync=False, reason="phase order")
                nc.vector.tensor_add(out=ht, in0=ht, in1=psum_c)
                nc.scalar.dma_start(out=out[i * P:(i + 1) * P, :], in_=ht)


def build_nc(debug: bool = False):
    nc = bacc.Bacc("TRN2", target_bir_lowering=False, debug=debug,
                   num_devices=B)
    embeds = nc.dram_tensor("embeds", [S, H], DT, kind="ExternalInput").ap()
    hidden = nc.dram_tensor("hidden", [S, H], DT, kind="ExternalInput").ap()
    w1t = nc.dram_tensor("w1t", [H, RH], BF, kind="ExternalInput").ap()
    b1t = nc.dram_tensor("b1t", [P, RB], DT, kind="ExternalInput").ap()
    w2tb = nc.dram_tensor("w2tb", [P, RB * N_SLOTS], DT,
                          kind="ExternalInput").ap()
    b2 = nc.dram_tensor("b2", [1, N_SLOTS], DT, kind="ExternalInput").ap()
    gate_r = nc.dram_tensor("gate_r", [P, KB], DT, kind="ExternalInput").ap()
    mem2 = nc.dram_tensor("mem2", [N_SLOTS * MEM, H], BF,
                          kind="ExternalInput").ap()
    maskd = nc.dram_tensor("maskd", [P, N_SLOTS], DT,
                           kind="ExternalInput").ap()
    out = nc.dram_tensor("out", [S, H], DT, kind="ExternalOutput").ap()

    with tile.TileContext(nc) as tc:
        build_kernel(tc, embeds, hidden, w1t, b1t, w2tb, b2, gate_r, mem2,
                     maskd, out)
    nc.compile()
    return nc


_NC_CACHE = None


def _get_nc():
    global _NC_CACHE
    if _NC_CACHE is None:
        _NC_CACHE = build_nc(debug=False)
    return _NC_CACHE


def make_in_maps(inputs: dict) -> list[dict]:
    embeds = np.asarray(inputs["embeds"], dtype=np.float32)
    hidden = np.asarray(inputs["hidden"], dtype=np.float32)
    W1 = np.asarray(inputs["W1"], dtype=np.float32)
    b1 = np.asarray(inputs["b1"], dtype=np.float32)
    W2 = np.asarray(inputs["W2"], dtype=np.float32)
    b2 = np.asarray(inputs["b2"], dtype=np.float32)
    gate_logits = np.asarray(inputs["gate_logits"], dtype=np.float32)
    memory = np.asarray(inputs["memory"], dtype=np.float32)
    import ml_dtypes

    shared = {
        "w1t": np.ascontiguousarray(W1.T).astype(ml_dtypes.bfloat16),
        "b1t": np.ascontiguousarray(b1.reshape(RB, P).T),
        "w2tb": np.ascontiguousarray(
            W2.T.reshape(RB, P, N_SLOTS).transpose(1, 0, 2).reshape(P, RB * N_SLOTS)
        ),
        "b2": np.ascontiguousarray(b2.reshape(1, N_SLOTS)),
        "gate_r": np.ascontiguousarray(gate_logits.T.reshape(KB, P).T),
        "mem2": np.ascontiguousarray(
            memory.transpose(1, 0, 2).reshape(N_SLOTS * MEM, H)
        ).astype(ml_dtypes.bfloat16),
        "maskd": np.ascontiguousarray(
            np.tile(np.eye(N_SLOTS, dtype=np.float32), (P // N_SLOTS, 1))
        ),
    }
    maps = []
    for c in range(B):
        m = dict(shared)
        m["embeds"] = np.ascontiguousarray(embeds[c])
        m["hidden"] = np.ascontiguousarray(hidden[c])
        maps.append(m)
    return maps


def run_spmd(inputs: dict, trace: bool = False):
    """Returns (output [B, S, H] float32, BassKernelResults)."""
    assert int(inputs.get("top_k", TOP_K)) == TOP_K
    nc = _get_nc()
    in_maps = make_in_maps(inputs)
    res = run_bass_kernel_spmd(nc, in_maps, core_ids=list(range(B)),
                               trace=trace)
    outs = np.stack([np.asarray(r["out"], dtype=np.float32)
                     for r in res.results])
    return outs, res


def kernel(**inputs) -> np.ndarray:
    outs, _ = run_spmd(inputs, trace=False)
    return outs
